# revision 22
# baseline (speedup 1.0000x reference)
"""Trainium2 Bass kernel for nn_MultiHeadAttention_68272800137483.

Linear attention (elu+1 feature map) with QKV projections and merge.
N=8 batch sharded one-batch-element-per-core across 8 NeuronCores.

Math (per batch element, derived from the reference with exact algebraic
simplifications):
  Q = q @ Wq.T + bq ; K = k @ Wk.T + bk            (bf16 matmuls)
  f(x) = elu(x)+1 = max(x+1, min(exp(x), 1))
  Qf = f(Q)  (q_mask deferred);  Kf = f(K) * kv_mask[s]
  KVraw[hd, j] = sum_s Kf[s, hd] * v[s, j];  Ksum[hd] = sum_s Kf[s, hd]
  KV[h,d,v] = sum_j KVraw[(h,d), j] Wv[(h,v), j] + bv[(h,v)] * Ksum[(h,d)]
     (this folds the V projection into the tiny KV matrix — v needs no
      projection, mask, or scaling; the /s and *s factors cancel)
  Zdot[l, h] = sum_d Qf[l,(h,d)] Ksum[(h,d)] + 1e30 * (1 - q_mask[l])
  Z = 1 / Zdot       (eps=1e-6 is negligible vs Zdot >= O(100); the 1e30
                      row makes masked-out query rows ~0 in the output)
  out[l, (h,v)] = (sum_d Qf[l,(h,d)] KV[h,d,v]) * Z[l, h]
  final = out @ Wm.T
"""

import numpy as np

import concourse.bacc as bacc
import concourse.mybir as mybir
import concourse.tile as tile
from concourse.bass_utils import run_bass_kernel_spmd
from concourse.masks import make_identity

F32 = mybir.dt.float32
F32R = mybir.dt.float32r
BF16 = mybir.dt.bfloat16
U8 = mybir.dt.uint8
AF = mybir.ActivationFunctionType
OP = mybir.AluOpType

L = S = 4096
E = 256
H = 8
D = 32
NCORES = 8
LCH = 512                     # l-chunk (one PSUM bank of fp32)
NCHUNK = L // LCH             # 8
NST = S // 128                # 32 s-tiles
BIG = 1.0e30


def build(debug_taps=False):
    nc = bacc.Bacc("TRN2", target_bir_lowering=False, debug=False,
                   num_devices=NCORES)

    q_d = nc.dram_tensor("q", [L, E], F32, kind="ExternalInput").ap()
    k_d = nc.dram_tensor("k", [S, E], F32, kind="ExternalInput").ap()
    v_d = nc.dram_tensor("v", [S, E], F32, kind="ExternalInput").ap()
    qm_d = nc.dram_tensor("q_mask", [L], U8, kind="ExternalInput").ap()
    km_d = nc.dram_tensor("kv_mask", [S], U8, kind="ExternalInput").ap()
    Wq_d = nc.dram_tensor("Wq", [E, E], F32, kind="ExternalInput").ap()
    bq_d = nc.dram_tensor("bq", [E], F32, kind="ExternalInput").ap()
    Wk_d = nc.dram_tensor("Wk", [E, E], F32, kind="ExternalInput").ap()
    bk_d = nc.dram_tensor("bk", [E], F32, kind="ExternalInput").ap()
    Wv_d = nc.dram_tensor("Wv", [E, E], F32, kind="ExternalInput").ap()
    bv_d = nc.dram_tensor("bv", [E], F32, kind="ExternalInput").ap()
    Wm_d = nc.dram_tensor("Wm", [E, E], F32, kind="ExternalInput").ap()
    out_d = nc.dram_tensor("out", [L, E], F32, kind="ExternalOutput").ap()
    dbg = {}
    if debug_taps:
        for nm, shp, dt in (("dKf", [128, NST, 256], F32R), ("dkT", [128, 2, S], BF16),
                            ("dqT", [128, 2, LCH], BF16), ("dKVraw", [128, 2, 256], F32),
                            ("dKsum", [128, 2, 1], F32), ("dbd", [128, 2, 128], F32R),
                            ("dZscat", [128, 2, 8], F32R), ("dQf", [128, 2, LCH], F32R),
                            ("dZ", [8, LCH], F32R), ("dZb", [128, 2, LCH], F32),
                            ("dnZ", [128, 2, LCH], F32R)):
            dbg[nm] = nc.dram_tensor(nm, shp, dt, kind="ExternalOutput").ap()

    with tile.TileContext(nc) as tc:
        with tc.tile_pool(name="setup", bufs=1) as su, \
             tc.tile_pool(name="persist", bufs=1) as pp, \
             tc.tile_pool(name="kstage", bufs=2) as ksg, \
             tc.tile_pool(name="vstage", bufs=2) as vsg, \
             tc.tile_pool(name="kelem", bufs=3) as kel, \
             tc.tile_pool(name="qstage", bufs=2) as qsg, \
             tc.tile_pool(name="qchunk", bufs=2) as qch:

            from contextlib import ExitStack
            _ps1 = ExitStack()
            psp = _ps1.enter_context(tc.tile_pool(name="ps_k1", bufs=2, space="PSUM"))
            pskv = _ps1.enter_context(tc.tile_pool(name="ps_kv", bufs=1, space="PSUM"))
            pse = _ps1.enter_context(tc.tile_pool(name="ps_tr", bufs=2, space="PSUM"))

            # ---------------- setup: constants ----------------
            ident = su.tile([128, 128], F32)
            make_identity(nc, ident)
            ones_f32 = su.tile([128, LCH], F32)
            nc.vector.memset(ones_f32, 1.0)
            zero_f32 = su.tile([128, 256], F32)
            nc.vector.memset(zero_f32, 0.0)
            ones_col = su.tile([128, 1], F32R)
            nc.vector.tensor_copy(out=ones_col, in_=ones_f32[:, 0:1])
            ones_row = su.tile([1, 128], F32R)
            nc.vector.tensor_copy(out=ones_row, in_=ones_f32[0:1, 0:128])
            ones8 = su.tile([1, 8], BF16)
            nc.vector.tensor_copy(out=ones8, in_=ones_f32[0:1, 0:8])
            ones512 = su.tile([1, LCH], F32R)
            nc.vector.tensor_copy(out=ones512, in_=ones_f32[0:1, :])
            minus1 = su.tile([128, 1], F32)
            nc.vector.memset(minus1, -1.0)

            # ---------------- setup: weights ----------------
            # WqT/WkT (bf16, [e_in, e_out] layout) via cast + DMA-transpose.
            wq_bf = su.tile([128, 2, 256], BF16, tag="wbf_q")
            wk_bf = su.tile([128, 2, 256], BF16, tag="wbf_k")
            WqT = pp.tile([128, 2, 256], BF16)
            WkT = pp.tile([128, 2, 256], BF16)
            for w_d, w_bf, wT in ((Wq_d, wq_bf, WqT), (Wk_d, wk_bf, WkT)):
                st = su.tile([128, 2, 256], F32, tag="wstage")
                for r in range(2):
                    nc.sync.dma_start(out=st[:, r, :], in_=w_d[128 * r:128 * (r + 1), :])
                nc.vector.tensor_copy(out=w_bf, in_=st)
                for mo in range(2):
                    nc.scalar.dma_start_transpose(
                        out=wT.rearrange("p ki o -> p ki o")[:, :, 128 * mo:128 * (mo + 1)],
                        in_=w_bf[:, mo, :])

            # WmT / WvT (f32r, [contract, out] layout) via PE transpose.
            WmT = pp.tile([128, 2, 256], F32R)
            WvT = pp.tile([128, 2, 256], F32R)
            for w_d, wT in ((Wm_d, WmT), (Wv_d, WvT)):
                st = su.tile([128, 2, 256], F32, tag="wstage")
                for r in range(2):
                    nc.sync.dma_start(out=st[:, r, :], in_=w_d[128 * r:128 * (r + 1), :])
                for ki in range(2):
                    for mo in range(2):
                        tp = pse.tile([128, 512], F32, tag="tr_ps")
                        nc.tensor.transpose(tp[:, :128], st[:, mo, 128 * ki:128 * (ki + 1)], ident)
                        nc.vector.tensor_copy(out=wT[:, ki, 128 * mo:128 * (mo + 1)],
                                              in_=tp[:, :128])

            # biases: rows [1, 256], +1 for the fused elu form
            bq1 = pp.tile([1, 256], F32R)
            bk1 = pp.tile([1, 256], F32R)
            for b_d, b1 in ((bq_d, bq1), (bk_d, bk1)):
                br = su.tile([1, 256], F32, tag="brow")
                nc.sync.dma_start(out=br, in_=b_d.unsqueeze(0))
                nc.vector.tensor_scalar(out=b1, in0=br, scalar1=1.0, scalar2=None,
                                        op0=OP.add)
            # bv broadcast to all partitions [128, 256] fp32
            bv_b = pp.tile([128, 256], F32)
            nc.sync.dma_start(out=bv_b, in_=bv_d.unsqueeze(0).partition_broadcast(128))

            # masks
            km_u8 = su.tile([32, 128], U8)
            nc.sync.dma_start(out=km_u8, in_=km_d.rearrange("(p c) -> p c", p=32))
            km_f = su.tile([32, 128], F32)
            nc.vector.tensor_copy(out=km_f, in_=km_u8)
            km_ps = pse.tile([128, 512], F32, tag="tr_ps")
            nc.tensor.transpose(km_ps[:, :32], km_f, ident[:32, :32])
            km_cols = pp.tile([128, 32], F32)
            nc.vector.tensor_copy(out=km_cols, in_=km_ps[:, :32])

            qm_u8 = su.tile([1, L], U8)
            nc.sync.dma_start(out=qm_u8, in_=qm_d.unsqueeze(0))
            qmBIG = pp.tile([1, L], BF16)
            nc.vector.tensor_scalar(out=qmBIG, in0=qm_u8, scalar1=-BIG,
                                    scalar2=BIG, op0=OP.mult, op1=OP.add)

            # Z-broadcast matrices: B8[p, f] = 1 where f == 32*p (f flat over
            # (hh, j)): head p of half hh=p//4 covers columns 32*(p%4) of that
            # half, i.e. flat 128*(p//4) + 32*(p%4) + [0,32) == 32*p + [0,32).
            B8f = su.tile([8, 256], F32, tag="B8f")
            nc.gpsimd.memset(B8f, 0.0)
            nc.gpsimd.affine_select(
                out=B8f.rearrange("p (a b) -> p a b", a=8), in_=B8f.rearrange("p (a b) -> p a b", a=8),
                compare_op=OP.not_equal, fill=1.0,
                base=0, pattern=[[-1, 8], [0, 32]], channel_multiplier=1)
            B8 = pp.tile([8, 2, 128], F32R)
            nc.vector.tensor_copy(out=B8, in_=B8f)

            # ---------------- K phase ----------------
            Kf = pp.tile([128, NST, 256], F32R)       # natural [s, hd]
            kT = pp.tile([128, 2, S], BF16)           # transposed k (bf16)
            kv_ps = []
            for h in range(2):
                kv_ps_h = pskv.tile([128, 258], F32, tag=f"kv{h}")
                kv_ps.append(kv_ps_h)

            NBLK = 4
            TPB = NST // NBLK  # 8 s-tiles per DMA block
            for blk in range(NBLK):
                kst = ksg.tile([128, TPB, 256], F32, tag="kst")
                nc.sync.dma_start(
                    out=kst, in_=k_d.rearrange("(b t p) e -> b p t e", b=NBLK, p=128)[blk])
                kbf = ksg.tile([128, TPB, 256], BF16, tag="kbf")
                nc.gpsimd.tensor_copy(out=kbf, in_=kst)
                vst = vsg.tile([128, TPB, 256], F32, tag="vst")
                nc.sync.dma_start(
                    out=vst, in_=v_d.rearrange("(b t p) e -> b p t e", b=NBLK, p=128)[blk])
                vr = vsg.tile([128, TPB, 258], F32R, tag="vr")
                nc.vector.tensor_copy(out=vr[:, :, 0:256], in_=vst)
                nc.vector.tensor_copy(out=vr[:, :, 256:258],
                                      in_=ones_f32[:, 0:2 * TPB].rearrange("p (t c) -> p t c", t=TPB))

                for t8 in range(TPB):
                    st_i = blk * TPB + t8
                    nc.scalar.dma_start_transpose(
                        out=kT[:, :, 128 * st_i:128 * (st_i + 1)],
                        in_=kbf[:, t8, :])
                    # projection: K_nat = kT.T @ WkT + (bk+1)
                    ps_k = psp.tile([128, 256], F32, tag="ps_k")
                    nc.tensor.matmul(ps_k, kT[:, 0, 128 * st_i:128 * (st_i + 1)],
                                     WkT[:, 0, :], start=True, stop=False)
                    nc.tensor.matmul(ps_k, kT[:, 1, 128 * st_i:128 * (st_i + 1)],
                                     WkT[:, 1, :], start=False, stop=False)
                    nc.tensor.matmul(ps_k, ones_row, bk1, start=False, stop=True)
                    # elu+1 (masked): Kf = max(Kb1*km, min(exp(Kb1-1),1)*km)
                    km_c = km_cols[:, st_i:st_i + 1]
                    e_bf = kel.tile([128, 256], BF16, tag="e_bf")
                    nc.scalar.activation(e_bf, ps_k, AF.Exp, bias=minus1)
                    e2m = kel.tile([128, 256], BF16, tag="e2m")
                    nc.gpsimd.tensor_scalar(out=e2m, in0=e_bf, scalar1=1.0,
                                            scalar2=km_c, op0=OP.min, op1=OP.mult)
                    nc.vector.scalar_tensor_tensor(
                        out=Kf[:, st_i, :], in0=ps_k, scalar=km_c, in1=e2m,
                        op0=OP.mult, op1=OP.max)
                    # KVraw accumulation + Ksum column
                    first = st_i == 0
                    last = st_i == NST - 1
                    for h in range(2):
                        kf_h = Kf[:, st_i, 128 * h:128 * (h + 1)]
                        nc.tensor.matmul(kv_ps[h], kf_h, vr[:, t8, :],
                                         start=first, stop=last)

            if debug_taps:
                nc.sync.dma_start(out=dbg["dKf"], in_=Kf)
                nc.sync.dma_start(out=dbg["dkT"], in_=kT)

            # ---------------- KV assembly ----------------
            KVraw = su.tile([128, 2, 256], F32)
            Ksum = su.tile([128, 2, 1], F32)
            for h in range(2):
                nc.vector.tensor_copy(out=KVraw[:, h, :], in_=kv_ps[h][:, 0:256])
                nc.vector.tensor_copy(out=Ksum[:, h, :], in_=kv_ps[h][:, 256:257])
            KVrawT = su.tile([128, 2, 256], F32R)
            for jh in range(2):
                for hh in range(2):
                    tp = pse.tile([128, 512], F32, tag="tr_ps")
                    nc.tensor.transpose(tp[:, :128], KVraw[:, hh, 128 * jh:128 * (jh + 1)],
                                        ident)
                    nc.vector.tensor_copy(out=KVrawT[:, jh, 128 * hh:128 * (hh + 1)],
                                          in_=tp[:, :128])
            # KVfull[hd, hv] = KVrawT.T @ WvT  (only diag blocks used)
            bd = pp.tile([128, 2, 128], F32R)         # block-diag KV per half
            nc.vector.tensor_copy(out=bd, in_=zero_f32)
            Zscat = pp.tile([128, 2, 8], F32R)        # Ksum scattered per head
            nc.vector.tensor_copy(out=Zscat, in_=zero_f32[:, 0:16])
            for hh in range(2):
                kvf = pse.tile([128, 512], F32, tag="tr_ps")
                nc.tensor.matmul(kvf[:, :256], KVrawT[:, 0, 128 * hh:128 * (hh + 1)],
                                 WvT[:, 0, :], start=True, stop=False)
                nc.tensor.matmul(kvf[:, :256], KVrawT[:, 1, 128 * hh:128 * (hh + 1)],
                                 WvT[:, 1, :], start=False, stop=True)
                for hl in range(4):
                    h = 4 * hh + hl
                    r = 32 * hl
                    nc.vector.scalar_tensor_tensor(
                        out=bd[r:r + 32, hh, r:r + 32],
                        in0=bv_b[r:r + 32, 32 * h:32 * h + 32],
                        scalar=Ksum[r:r + 32, hh, :].squeeze(-1).unsqueeze(1),
                        in1=kvf[r:r + 32, 32 * h:32 * h + 32],
                        op0=OP.mult, op1=OP.add)
                    nc.vector.tensor_copy(out=Zscat[r:r + 32, hh, h:h + 1],
                                          in_=Ksum[r:r + 32, hh, :])

            if debug_taps:
                for nm, t in (("dKVraw", KVraw), ("dKsum", Ksum)):
                    nc.sync.dma_start(out=dbg[nm], in_=t)
                for nm, t in (("dbd", bd), ("dZscat", Zscat)):
                    nc.sync.dma_start(out=dbg[nm], in_=t)

            _ps1.close()
            _ps2 = ExitStack()
            psq_pool = _ps2.enter_context(tc.tile_pool(name="ps_q", bufs=2, space="PSUM"))
            psb_pool = _ps2.enter_context(tc.tile_pool(name="ps_b", bufs=2, space="PSUM"))
            psz_pool = _ps2.enter_context(tc.tile_pool(name="ps_z", bufs=1, space="PSUM"))
            psn_pool = _ps2.enter_context(tc.tile_pool(name="ps_n", bufs=2, space="PSUM"))
            psm_pool = _ps2.enter_context(tc.tile_pool(name="ps_m", bufs=1, space="PSUM"))

            # ---------------- Q + einsum phase ----------------
            for c in range(NCHUNK):
                qst = qsg.tile([128, 4, 256], F32, tag="qst")
                nc.sync.dma_start(
                    out=qst,
                    in_=q_d.rearrange("(b t p) e -> b p t e", b=NCHUNK, p=128)[c])
                qbf = qsg.tile([128, 4, 256], BF16, tag="qbf")
                nc.gpsimd.tensor_copy(out=qbf, in_=qst)
                qT = qch.tile([128, 2, LCH], BF16, tag="qT")
                for lt in range(4):
                    nc.scalar.dma_start_transpose(
                        out=qT[:, :, 128 * lt:128 * (lt + 1)],
                        in_=qbf[:, lt, :])
                # projection + elu -> Qf (f32r, transposed [hd, l])
                Qf = qch.tile([128, 2, LCH], F32R, tag="Qf")
                ps_q = []
                for ho in range(2):
                    ps_q_h = psq_pool.tile([128, LCH], F32, tag="ps_q")
                    ps_q.append(ps_q_h)
                for ho in range(2):
                    nc.tensor.matmul(ps_q[ho], WqT[:, 0, 128 * ho:128 * (ho + 1)],
                                     qT[:, 0, :], start=True, stop=False)
                    nc.tensor.matmul(ps_q[ho], WqT[:, 1, 128 * ho:128 * (ho + 1)],
                                     qT[:, 1, :], start=False, stop=False)
                    nc.tensor.matmul(ps_q[ho], bq1[:, 128 * ho:128 * (ho + 1)],
                                     ones512, start=False, stop=True)
                    e_bf = qch.tile([128, LCH], BF16, tag="qe_bf")
                    nc.scalar.activation(e_bf, ps_q[ho], AF.Exp, bias=minus1)
                    e2 = qch.tile([128, LCH], BF16, tag="qe2")
                    nc.gpsimd.tensor_scalar(out=e2, in0=e_bf, scalar1=1.0,
                                            scalar2=None, op0=OP.min)
                    nc.vector.tensor_tensor(out=Qf[:, ho, :], in0=ps_q[ho],
                                            in1=e2, op=OP.max)
                if debug_taps and c == 0:
                    nc.sync.dma_start(out=dbg["dqT"], in_=qT)
                    nc.sync.dma_start(out=dbg["dQf"], in_=Qf)
                # Zdot -> Z
                zd = psz_pool.tile([8, LCH], F32, tag="zd")
                nc.tensor.matmul(zd, Zscat[:, 0, :], Qf[:, 0, :], start=True, stop=False)
                nc.tensor.matmul(zd, Zscat[:, 1, :], Qf[:, 1, :], start=False, stop=False)
                nc.tensor.matmul(zd, ones8, qmBIG[:, c * LCH:(c + 1) * LCH], start=False, stop=True)
                Z = qch.tile([8, LCH], F32R, tag="Z")
                with nc.allow_low_precision(reason="f32r Z for PE broadcast"):
                    nc.vector.reciprocal(Z, zd)
                Zb = qch.tile([128, 2, LCH], F32, tag="Zb")
                for hh in range(2):
                    zb_ps = psb_pool.tile([128, LCH], F32, tag="zb")
                    nc.tensor.matmul(zb_ps, B8[:, hh, :], Z, start=True, stop=True)
                    nc.scalar.copy(Zb[:, hh, :], zb_ps)
                if debug_taps and c == 0:
                    nc.sync.dma_start(out=dbg["dZ"], in_=Z)
                    nc.sync.dma_start(out=dbg["dZb"], in_=Zb)
                # numer = bd.T @ Qf ; apply Z ; merge
                nZ = qch.tile([128, 2, LCH], F32R, tag="nZ")
                for hh in range(2):
                    nm = psn_pool.tile([128, LCH], F32, tag="nm")
                    nc.tensor.matmul(nm, bd[:, hh, :], Qf[:, hh, :],
                                     start=True, stop=True)
                    nc.vector.tensor_tensor(out=nZ[:, hh, :], in0=nm,
                                            in1=Zb[:, hh, :], op=OP.mult)
                if debug_taps and c == 0:
                    nc.sync.dma_start(out=dbg["dnZ"], in_=nZ)
                mg_sb = qch.tile([128, 4, 256], F32, tag="mg_sb")
                for lt in range(4):
                    mg = psm_pool.tile([128, 256], F32, tag="mg")
                    nc.tensor.matmul(mg, nZ[:, 0, 128 * lt:128 * (lt + 1)],
                                     WmT[:, 0, :], start=True, stop=False)
                    nc.tensor.matmul(mg, nZ[:, 1, 128 * lt:128 * (lt + 1)],
                                     WmT[:, 1, :], start=False, stop=True)
                    nc.scalar.copy(mg_sb[:, lt, :], mg)
                nc.sync.dma_start(
                    out=out_d.rearrange("(c t p) e -> c p t e", c=NCHUNK, p=128)[c],
                    in_=mg_sb)
            _ps2.close()

    nc.compile()
    return nc


_NC = None


def kernel(**inputs) -> np.ndarray:
    global _NC
    if _NC is None:
        _NC = build()
    nc = _NC

    q = np.ascontiguousarray(np.asarray(inputs["q"], dtype=np.float32))
    k = np.ascontiguousarray(np.asarray(inputs["k"], dtype=np.float32))
    v = np.ascontiguousarray(np.asarray(inputs["v"], dtype=np.float32))
    qm = np.asarray(inputs["q_mask"]).astype(np.uint8)
    km = np.asarray(inputs["kv_mask"]).astype(np.uint8)
    shared = {
        "Wq": np.ascontiguousarray(np.asarray(inputs["Wq"], dtype=np.float32)),
        "bq": np.ascontiguousarray(np.asarray(inputs["bq"], dtype=np.float32)),
        "Wk": np.ascontiguousarray(np.asarray(inputs["Wk"], dtype=np.float32)),
        "bk": np.ascontiguousarray(np.asarray(inputs["bk"], dtype=np.float32)),
        "Wv": np.ascontiguousarray(np.asarray(inputs["Wv"], dtype=np.float32)),
        "bv": np.ascontiguousarray(np.asarray(inputs["bv"], dtype=np.float32)),
        "Wm": np.ascontiguousarray(np.asarray(inputs["Wm"], dtype=np.float32)),
    }
    in_maps = []
    for c in range(NCORES):
        m = {"q": q[c], "k": k[c], "v": v[c], "q_mask": qm[c], "kv_mask": km[c]}
        m.update(shared)
        in_maps.append(m)

    res = run_bass_kernel_spmd(nc, in_maps, core_ids=list(range(NCORES)))
    return np.stack([res.results[c]["out"] for c in range(NCORES)], axis=0)


def _make_in_maps(inputs):
    q = np.ascontiguousarray(np.asarray(inputs["q"], dtype=np.float32))
    k = np.ascontiguousarray(np.asarray(inputs["k"], dtype=np.float32))
    v = np.ascontiguousarray(np.asarray(inputs["v"], dtype=np.float32))
    qm = np.asarray(inputs["q_mask"]).astype(np.uint8)
    km = np.asarray(inputs["kv_mask"]).astype(np.uint8)
    shared = {n: np.ascontiguousarray(np.asarray(inputs[n], dtype=np.float32))
              for n in ("Wq", "bq", "Wk", "bk", "Wv", "bv", "Wm")}
    in_maps = []
    for c in range(NCORES):
        m = {"q": q[c], "k": k[c], "v": v[c], "q_mask": qm[c], "kv_mask": km[c]}
        m.update(shared)
        in_maps.append(m)
    return in_maps


def bench(iters=20, REPEATS=16, **inputs):
    """Time repeated NEFF executions with inputs pre-staged on device.

    Returns (min_ns, all_ns). Includes per-call axon dispatch overhead,
    so it is an upper bound on device exec time.
    """
    import time
    import jax
    import jax.numpy as jnp
    from jax.sharding import Mesh, PartitionSpec
    from jax.experimental.shard_map import shard_map
    from concourse import bass2jax

    global _NC
    if _NC is None:
        _NC = build()
    nc = _NC
    bass2jax.install_neuronx_cc_hook()

    in_maps = _make_in_maps(inputs)
    import concourse.mybir as _mb
    in_names, out_names, out_avals = [], [], []
    for alloc in nc.m.functions[0].allocations:
        if not isinstance(alloc, _mb.MemoryLocationSet):
            continue
        name = alloc.memorylocations[0].name
        if alloc.kind == "ExternalInput":
            in_names.append(name)
        elif alloc.kind == "ExternalOutput":
            out_names.append(name)
            out_avals.append(jax.core.ShapedArray(tuple(alloc.tensor_shape),
                                                  _mb.dt.np(alloc.dtype)))
    pname = nc.partition_id_tensor.name if nc.partition_id_tensor else None
    if pname in in_names:
        in_names.remove(pname)
    n_params = len(in_names)
    all_names = in_names + out_names + ([pname] if pname else [])

    def _make_body(repeats):
        def _body(*args):
            params = list(args[:n_params])
            outs = list(args[n_params:])
            for _ in range(repeats):
                ops = params + outs
                if pname:
                    ops.append(bass2jax.partition_id_tensor())
                outs = list(bass2jax._bass_exec_p.bind(
                    *ops, out_avals=tuple(out_avals), in_names=tuple(all_names),
                    out_names=tuple(out_names), lowering_input_output_aliases=(),
                    sim_require_finite=True, sim_require_nnan=True, nc=nc))
            return tuple(outs)
        return _body

    devices = jax.devices()[:NCORES]
    mesh = Mesh(np.asarray(devices), ("core",))
    nin = n_params + len(out_names)
    sharded = jax.jit(shard_map(_make_body(1), mesh=mesh,
                                in_specs=(PartitionSpec("core"),) * nin,
                                out_specs=(PartitionSpec("core"),) * len(out_names),
                                check_rep=False), keep_unused=True)
    concat_in = [np.concatenate([in_maps[c][nm] for c in range(NCORES)], axis=0)
                 for nm in in_names]
    concat_zero = [np.zeros((NCORES * a.shape[0], *a.shape[1:]), a.dtype)
                   for a in out_avals]
    from jax.sharding import NamedSharding
    shard = NamedSharding(mesh, PartitionSpec("core"))
    dev_in = [jax.device_put(x, shard) for x in concat_in]
    dev_zero = [jax.device_put(x, shard) for x in concat_zero]
    # warmup (also triggers compile)
    out = sharded(*dev_in, *dev_zero)
    jax.block_until_ready(out)

    def run_queue(m):
        t0 = time.perf_counter()
        outs = out
        for _ in range(m):
            outs = sharded(*dev_in, *(outs if CHAIN else dev_zero))
        jax.block_until_ready(outs)
        return (time.perf_counter() - t0) * 1e9

    CHAIN = True   # feed outputs back as next call's donate buffers (serializes)
    t1 = min(run_queue(1) for _ in range(iters))
    tR = min(run_queue(REPEATS) for _ in range(iters))
    per_iter = (tR - t1) / (REPEATS - 1)
    return per_iter, ([t1], [tR])


def profile(**inputs):
    """Run once with NTFF tracing; returns (exec_time_ns, trace_path)."""
    global _NC
    if _NC is None:
        _NC = build()
    res = run_bass_kernel_spmd(_NC, _make_in_maps(inputs),
                               core_ids=list(range(NCORES)), trace=True)
    trace_path = None
    if res.instructions_and_trace is not None:
        trace_path = res.instructions_and_trace[1]
    return res.exec_time_ns, trace_path
